# revision 17
# baseline (speedup 1.0000x reference)
"""Trainium2 Bass kernel for DWConvBlock3D:
depthwise 3x3x3 conv (pad 1) + InstanceNorm3d + ReLU on x:(2,64,128,128) f32.

Strategy (8 NeuronCores, channel sharding => zero communication):
  - Each core owns 8 channels x 2 batches = 16 (b,c) "pairs".
  - Winograd F(2,3) along D, computed host-side: the host ships 4 transform
    points z_i[t] (t = d-tile of 2 output slices) instead of x; on device each
    point needs only a 2D (kh,kw) conv: kh rides the H-band matmul (H=128 on
    partitions), kw is a free-dim shift => 4 points x 3 kw = 12 matmul passes
    per 8 output d-slices vs 18 for direct conv (PE column stream is the
    bottleneck; this cuts it 1.5x).
  - Per superchunk (4 d-tiles = 8 output d): points m0..m3 accumulate in 4
    PSUM banks; inverse transform y_even = m0+m1+m2, y_odd = m1-m2-m3 runs on
    VectorE (4 ops, fp32 PSUM -> fp16 y), with sum(y) free via accum_out.
  - InstanceNorm: sum from inverse-transform accum_out; sum(y^2) via one
    ScalarE Square-activation pass (accum_out, result dumped into the dead z
    tile); cross-partition reduction via GpSimd partition_all_reduce.
  - Final (y*scale+bias, ReLU) is ScalarE activation with per-partition
    scale/bias; output fp16 (cast to fp32 on host).
"""

import sys

if "/opt/trn_rl_repo" not in sys.path:
    sys.path.insert(0, "/opt/trn_rl_repo")

import numpy as np

B, C, D, H, W = 2, 64, 64, 128, 128
N_CORES = 8
CH_PER_CORE = C // N_CORES  # 8
N_PAIRS = B * CH_PER_CORE  # 16
WP = W + 2  # host-padded W (zero borders) -> free w-shifts
NT = D // 2  # 32 winograd tiles (2 output d-slices each)
SC_T = 4  # d-tiles per superchunk (4 tiles = 8 output d = 1 PSUM bank/point)
N_SC = NT // SC_T  # 8
FREE = D * W  # 8192 free elements per partition per pair
NV = D * H * W  # normalization element count per (b,c)
EPS = 1e-5


def build_program(n_pairs=N_PAIRS, ch_per_core=CH_PER_CORE):
    import concourse.bacc as bacc
    import concourse.mybir as mybir
    from concourse import bass_isa
    from concourse.tile import TileContext

    f32 = mybir.dt.float32
    f16 = mybir.dt.float16
    nc = bacc.Bacc("TRN2", target_bir_lowering=False, debug=False, num_devices=N_CORES)

    zs = nc.dram_tensor("zs", [n_pairs, H, 4, NT, WP], f16, kind="ExternalInput")
    bands = nc.dram_tensor(
        "bands", [H, ch_per_core, 4, 3, H], f16, kind="ExternalInput"
    )  # [h_in, ci, point, kw, h_out]
    gb = nc.dram_tensor("gb", [128, 2 * n_pairs], f32, kind="ExternalInput")
    out = nc.dram_tensor("out", [n_pairs, H, FREE], f16, kind="ExternalOutput")

    with TileContext(nc) as tc:
        with (
            tc.tile_pool(name="singles", bufs=1) as singles,
            tc.tile_pool(name="zp", bufs=3) as zpool,
            tc.tile_pool(name="yp", bufs=3) as ypool,
            tc.tile_pool(name="st", bufs=3) as stats,
            tc.tile_pool(name="tp", bufs=4) as tpool,
            tc.tile_pool(name="psmm", bufs=8, space="PSUM") as psum_mm,
        ):
            band_sb = singles.tile([H, ch_per_core, 4, 3, H], f16)
            gb_sb = singles.tile([128, 2 * n_pairs], f32)
            nc.sync.dma_start(out=gb_sb[:], in_=gb[:])

            for p in range(n_pairs):
                ci = p % ch_per_core

                zt = zpool.tile([H, 4, NT, WP], f16, tag="zt")
                nc.sync.dma_start(out=zt[:], in_=zs[p])
                if p < ch_per_core:
                    # just-in-time per-channel band load (keeps startup short)
                    nc.sync.dma_start(out=band_sb[:, ci], in_=bands[:, ci])

                # d = 2*t + parity: [H, t, parity, w] so even/odd outputs are
                # plain dim slices (memory layout identical to [H, D, W])
                y = ypool.tile([H, NT, 2, W], f16, tag="y")
                sums = stats.tile([128, 2 * N_SC], f32, tag="sums")
                st2 = stats.tile([128, 3], f32, tag="st2")

                for sc in range(N_SC):
                    t0 = sc * SC_T
                    m = [
                        psum_mm.tile([128, SC_T, W], f32, tag="mm", name=f"mm_{p}_{sc}_{i}")
                        for i in range(4)
                    ]
                    s1 = tpool.tile([128, SC_T, W], f32, tag="s1")
                    s2 = tpool.tile([128, SC_T, W], f32, tag="s2")
                    pre = tpool.tile([128, SC_T, W], f32, tag="pre")
                    q = tpool.tile([128, SC_T, W], f32, tag="q")

                    def mm_point(i):
                        for kw in range(3):
                            nc.tensor.matmul(
                                m[i][:],
                                band_sb[:, ci, i, kw, :],
                                zt[:, i, t0 : t0 + SC_T, kw : kw + W],
                                start=(kw == 0),
                                stop=(kw == 2),
                                skip_group_check=True,
                            )

                    # engines may read only ONE psum operand per op: ScalarE
                    # evicts m1/m2, DVE combines; interleaved with the matmuls
                    mm_point(1)
                    mm_point(2)
                    nc.scalar.activation(
                        out=s1[:], in_=m[1][:], func=mybir.ActivationFunctionType.Copy
                    )
                    nc.scalar.activation(
                        out=s2[:], in_=m[2][:], func=mybir.ActivationFunctionType.Copy
                    )
                    mm_point(0)
                    nc.gpsimd.tensor_add(pre[:], s1[:], s2[:])
                    nc.gpsimd.tensor_sub(q[:], s1[:], s2[:])
                    # y_even = m0 + (m1+m2) ; y_odd = (m1-m2) - m3
                    nc.vector.scalar_tensor_tensor(
                        out=y[:, t0 : t0 + SC_T, 0, :],
                        in0=m[0][:], scalar=1.0, in1=pre[:],
                        op0=mybir.AluOpType.mult, op1=mybir.AluOpType.add,
                        accum_out=sums[:, 2 * sc : 2 * sc + 1],
                    )
                    mm_point(3)
                    nc.vector.scalar_tensor_tensor(
                        out=y[:, t0 : t0 + SC_T, 1, :],
                        in0=m[3][:], scalar=-1.0, in1=q[:],
                        op0=mybir.AluOpType.mult, op1=mybir.AluOpType.add,
                        accum_out=sums[:, 2 * sc + 1 : 2 * sc + 2],
                    )

                # ---- per-partition stats
                nc.vector.tensor_reduce(
                    out=st2[:, 0:1], in_=sums[:], axis=mybir.AxisListType.X,
                    op=mybir.AluOpType.add,
                )
                # sum(y^2): half on ScalarE (Square), half on DVE (stt); squares
                # dumped into the dead z tile
                yf = y[:].rearrange("p a b c -> p (a b c)")
                hq = FREE // 2
                sq_scratch = zt[:].rearrange("p a b c -> p (a b c)")
                nc.scalar.activation(
                    out=sq_scratch[:, 0:hq], in_=yf[:, 0:hq],
                    func=mybir.ActivationFunctionType.Square,
                    accum_out=st2[:, 1:2],
                )
                nc.vector.scalar_tensor_tensor(
                    out=sq_scratch[:, hq:FREE], in0=yf[:, hq:FREE], scalar=1.0,
                    in1=yf[:, hq:FREE],
                    op0=mybir.AluOpType.mult, op1=mybir.AluOpType.mult,
                    accum_out=st2[:, 2:3],
                )

                # ---- all-reduce across partitions (GpSimd) -> every partition
                # holds (sum, sumsq); the stats math then runs replicated
                ast = stats.tile([128, 3], f32, tag="ast")
                nc.gpsimd.partition_all_reduce(
                    ast[:], st2[:], 128, bass_isa.ReduceOp.add
                )

                sm = stats.tile([128, 10], f32, tag="sm")
                mean, ex2 = sm[:, 0:1], sm[:, 1:2]
                msq, vpe = sm[:, 2:3], sm[:, 3:4]
                std, r0 = sm[:, 4:5], sm[:, 5:6]
                t1, t2 = sm[:, 6:7], sm[:, 7:8]
                t4, rr = sm[:, 8:9], sm[:, 9:10]
                nc.vector.tensor_scalar_mul(mean, ast[:, 0:1], 1.0 / NV)
                nc.vector.tensor_add(ex2, ast[:, 1:2], ast[:, 2:3])
                nc.vector.tensor_scalar_mul(ex2, ex2, 1.0 / NV)
                nc.vector.tensor_mul(msq, mean, mean)
                nc.vector.tensor_sub(vpe, ex2, msq)
                nc.vector.tensor_scalar_add(vpe, vpe, EPS)
                nc.scalar.activation(std, vpe, mybir.ActivationFunctionType.Sqrt)
                nc.vector.reciprocal(r0, std)
                # one Newton step: r = r0*(1.5 - 0.5*vpe*r0^2)
                nc.vector.tensor_mul(t1, r0, r0)
                nc.vector.tensor_mul(t2, t1, vpe)
                nc.vector.tensor_scalar(
                    t4, t2, -0.5, 1.5, op0=mybir.AluOpType.mult, op1=mybir.AluOpType.add
                )
                nc.vector.tensor_mul(rr, r0, t4)

                sb2 = stats.tile([128, 2], f32, tag="sb2")
                sc_, bi = sb2[:, 0:1], sb2[:, 1:2]
                # scale = gamma * rstd ; bias = beta - mean*scale
                nc.vector.tensor_mul(sc_, rr, gb_sb[:, p : p + 1])
                nc.vector.tensor_mul(t1, mean, sc_)
                nc.vector.tensor_sub(bi, gb_sb[:, n_pairs + p : n_pairs + p + 1], t1)

                # ---- fused normalize + ReLU (in place), then store.
                # split halves so the ScalarE apply overlaps the out-DMA
                hf = FREE // 2
                for h2 in range(2):
                    ysl = yf[:, h2 * hf : (h2 + 1) * hf]
                    nc.scalar.activation(
                        out=ysl,
                        in_=ysl,
                        func=mybir.ActivationFunctionType.Relu,
                        scale=sc_,
                        bias=bi,
                    )
                    nc.gpsimd.dma_start(
                        out=out[p][:, h2 * hf : (h2 + 1) * hf], in_=ysl
                    )

    nc.compile()
    return nc


_NC_CACHE = None


def _get_program():
    global _NC_CACHE
    if _NC_CACHE is None:
        _NC_CACHE = build_program()
    return _NC_CACHE


# Winograd F(2,3) kernel transform along kd
_G = np.array(
    [[1.0, 0.0, 0.0], [0.5, 0.5, 0.5], [0.5, -0.5, 0.5], [0.0, 0.0, 1.0]], np.float32
)


def make_core_inputs(x, w, gamma, beta, core):
    cs = slice(CH_PER_CORE * core, CH_PER_CORE * (core + 1))
    # (b, ci, d, h, w) -> pairs x (h, d, w), pair = b*8+ci; d/w zero-padded
    xp = np.zeros((N_PAIRS, D + 2, H, WP), np.float32)
    xp[:, 1 : D + 1, :, 1 : W + 1] = x[:, cs].reshape(N_PAIRS, D, H, W)
    # input transform: z_i[t] from window xpad[2t .. 2t+3] (= x[2t-1 .. 2t+2])
    a0 = xp[:, 0 : 2 * NT : 2]      # x(2t-1)
    a1 = xp[:, 1 : 2 * NT + 1 : 2]  # x(2t)
    a2 = xp[:, 2 : 2 * NT + 2 : 2]  # x(2t+1)
    a3 = xp[:, 3 : 2 * NT + 3 : 2]  # x(2t+2)
    z = np.empty((N_PAIRS, 4, NT, H, WP), np.float32)
    z[:, 0] = a0 - a2
    z[:, 1] = a1 + a2
    z[:, 2] = a2 - a1
    z[:, 3] = a1 - a3
    # -> [pair, H, point, t, WP]
    zc = np.ascontiguousarray(z.transpose(0, 3, 1, 2, 4)).astype(np.float16)

    # bands from the kd-transformed kernels ghat[i, kh, kw]
    bands = np.zeros((H, CH_PER_CORE, 4, 3, H), np.float32)
    eye0 = np.eye(H, dtype=np.float32)
    eyep = np.eye(H, k=1, dtype=np.float32)  # B[h-1, h]: kh=0 tap
    eyem = np.eye(H, k=-1, dtype=np.float32)  # B[h+1, h]: kh=2 tap
    for ci in range(CH_PER_CORE):
        c = CH_PER_CORE * core + ci
        ghat = np.einsum("ik,khw->ihw", _G, w[c, 0])  # [4, 3(kh), 3(kw)]
        for i in range(4):
            for kw in range(3):
                wk = ghat[i, :, kw]
                bands[:, ci, i, kw, :] = wk[0] * eyep + wk[1] * eye0 + wk[2] * eyem
    gbv = np.broadcast_to(
        np.concatenate([np.tile(gamma[cs], B), np.tile(beta[cs], B)])
        .astype(np.float32)
        .reshape(1, 2 * N_PAIRS),
        (128, 2 * N_PAIRS),
    ).copy()
    return {"zs": zc, "bands": bands.astype(np.float16), "gb": gbv}


def kernel(x, w, gamma, beta):
    from concourse.bass_utils import run_bass_kernel_spmd

    x = np.asarray(x, dtype=np.float32)
    w = np.asarray(w, dtype=np.float32)
    gamma = np.asarray(gamma, dtype=np.float32)
    beta = np.asarray(beta, dtype=np.float32)

    nc = _get_program()
    in_maps = [make_core_inputs(x, w, gamma, beta, k) for k in range(N_CORES)]
    res = run_bass_kernel_spmd(nc, in_maps, core_ids=list(range(N_CORES)))

    out = np.empty((B, C, D, H, W), np.float32)
    for k in range(N_CORES):
        cs = slice(CH_PER_CORE * k, CH_PER_CORE * (k + 1))
        yc = res.results[k]["out"].astype(np.float32).reshape(B, CH_PER_CORE, H, D, W)
        out[:, cs] = yc.transpose(0, 1, 3, 2, 4)
    return out


# revision 20
# speedup vs baseline: 1.7857x; 1.7857x over previous
"""Trainium2 Bass kernel for DWConvBlock3D:
depthwise 3x3x3 conv (pad 1) + InstanceNorm3d + ReLU on x:(2,64,128,128) f32.

Strategy (8 NeuronCores, channel sharding => zero communication):
  - Each core owns 8 channels x 2 batches = 16 (b,c) "pairs".
  - Winograd F(2,3) along D, computed host-side: the host ships 4 transform
    points z_i[t] (t = d-tile of 2 output slices) instead of x; on device each
    point needs only a 2D (kh,kw) conv: kh rides the H-band matmul (H=128 on
    partitions), kw is a free-dim shift => 4 points x 3 kw = 12 matmul passes
    per 8 output d-slices vs 18 for direct conv (PE column stream is the
    bottleneck; this cuts it 1.5x).
  - Per superchunk (4 d-tiles = 8 output d): points m0..m3 accumulate in 4
    PSUM banks; inverse transform y_even = m0+m1+m2, y_odd = m1-m2-m3 runs on
    VectorE (4 ops, fp32 PSUM -> fp16 y), with sum(y) free via accum_out.
  - InstanceNorm: sum from inverse-transform accum_out; sum(y^2) via one
    ScalarE Square-activation pass (accum_out, result dumped into the dead z
    tile); cross-partition reduction via GpSimd partition_all_reduce.
  - Final (y*scale+bias, ReLU) is ScalarE activation with per-partition
    scale/bias; output fp16 (cast to fp32 on host).
"""

import sys

if "/opt/trn_rl_repo" not in sys.path:
    sys.path.insert(0, "/opt/trn_rl_repo")

import numpy as np

B, C, D, H, W = 2, 64, 64, 128, 128
N_CORES = 8
CH_PER_CORE = C // N_CORES  # 8
N_PAIRS = B * CH_PER_CORE  # 16
WP = W + 2  # host-padded W (zero borders) -> free w-shifts
NT = D // 2  # 32 winograd tiles (2 output d-slices each)
SC_T = 4  # d-tiles per superchunk (4 tiles = 8 output d = 1 PSUM bank/point)
N_SC = NT // SC_T  # 8
FREE = D * W  # 8192 free elements per partition per pair
NV = D * H * W  # normalization element count per (b,c)
EPS = 1e-5


def build_program(n_pairs=N_PAIRS, ch_per_core=CH_PER_CORE):
    import concourse.bacc as bacc
    import concourse.mybir as mybir
    from concourse import bass_isa
    from concourse.tile import TileContext

    f32 = mybir.dt.float32
    f16 = mybir.dt.float16
    nc = bacc.Bacc("TRN2", target_bir_lowering=False, debug=False, num_devices=N_CORES)

    zs = nc.dram_tensor("zs", [n_pairs, H, 4, NT, WP], f16, kind="ExternalInput")
    bands = nc.dram_tensor(
        "bands", [H, ch_per_core, 4, 3, H], f16, kind="ExternalInput"
    )  # [h_in, ci, point, kw, h_out]
    gb = nc.dram_tensor("gb", [128, 2 * n_pairs], f32, kind="ExternalInput")
    out = nc.dram_tensor("out", [n_pairs, H, FREE], f16, kind="ExternalOutput")

    with TileContext(nc) as tc:
        with (
            tc.tile_pool(name="singles", bufs=1) as singles,
            tc.tile_pool(name="zp", bufs=3) as zpool,
            tc.tile_pool(name="yp", bufs=3) as ypool,
            tc.tile_pool(name="st", bufs=3) as stats,
            tc.tile_pool(name="tp", bufs=4) as tpool,
            tc.tile_pool(name="psmm", bufs=8, space="PSUM") as psum_mm,
        ):
            band_sb = singles.tile([H, ch_per_core, 4, 3, H], f16)
            gb_sb = singles.tile([128, 2 * n_pairs], f32)
            nc.sync.dma_start(out=gb_sb[:], in_=gb[:])

            for p in range(n_pairs):
                ci = p % ch_per_core

                zt = zpool.tile([H, 4, NT, WP], f16, tag="zt")
                nc.sync.dma_start(out=zt[:], in_=zs[p])
                if p < ch_per_core:
                    # just-in-time per-channel band load (keeps startup short)
                    nc.sync.dma_start(out=band_sb[:, ci], in_=bands[:, ci])

                # d = 2*t + parity: [H, t, parity, w] so even/odd outputs are
                # plain dim slices (memory layout identical to [H, D, W])
                y = ypool.tile([H, NT, 2, W], f16, tag="y")
                sums = stats.tile([128, 2 * N_SC], f32, tag="sums")
                st2 = stats.tile([128, 3], f32, tag="st2")

                for sc in range(N_SC):
                    t0 = sc * SC_T
                    m = [
                        psum_mm.tile([128, SC_T, W], f32, tag="mm", name=f"mm_{p}_{sc}_{i}")
                        for i in range(4)
                    ]
                    s1 = tpool.tile([128, SC_T, W], f32, tag="s1")
                    u = tpool.tile([128, SC_T, W], f32, tag="u")
                    v = tpool.tile([128, SC_T, W], f32, tag="v")

                    def mm_point(i):
                        for kw in range(3):
                            nc.tensor.matmul(
                                m[i][:],
                                band_sb[:, ci, i, kw, :],
                                zt[:, i, t0 : t0 + SC_T, kw : kw + W],
                                start=(kw == 0),
                                stop=(kw == 2),
                                skip_group_check=True,
                            )

                    # engines may read only ONE psum operand per op: ScalarE
                    # evicts m1 -> s1, then DVE chains (m2 is read twice):
                    #   u = m2 + s1 = m1+m2 ; v = -2*m2 + u = m1-m2
                    #   y_even = m0 + u ; y_odd = -m3 + v
                    mm_point(1)
                    mm_point(2)
                    nc.scalar.activation(
                        out=s1[:], in_=m[1][:], func=mybir.ActivationFunctionType.Copy
                    )
                    nc.vector.scalar_tensor_tensor(
                        out=u[:], in0=m[2][:], scalar=1.0, in1=s1[:],
                        op0=mybir.AluOpType.mult, op1=mybir.AluOpType.add,
                    )
                    mm_point(0)
                    nc.vector.scalar_tensor_tensor(
                        out=v[:], in0=m[2][:], scalar=-2.0, in1=u[:],
                        op0=mybir.AluOpType.mult, op1=mybir.AluOpType.add,
                    )
                    nc.vector.scalar_tensor_tensor(
                        out=y[:, t0 : t0 + SC_T, 0, :],
                        in0=m[0][:], scalar=1.0, in1=u[:],
                        op0=mybir.AluOpType.mult, op1=mybir.AluOpType.add,
                        accum_out=sums[:, 2 * sc : 2 * sc + 1],
                    )
                    mm_point(3)
                    nc.vector.scalar_tensor_tensor(
                        out=y[:, t0 : t0 + SC_T, 1, :],
                        in0=m[3][:], scalar=-1.0, in1=v[:],
                        op0=mybir.AluOpType.mult, op1=mybir.AluOpType.add,
                        accum_out=sums[:, 2 * sc + 1 : 2 * sc + 2],
                    )

                # ---- per-partition stats
                nc.vector.tensor_reduce(
                    out=st2[:, 0:1], in_=sums[:], axis=mybir.AxisListType.X,
                    op=mybir.AluOpType.add,
                )
                # sum(y^2): half on ScalarE (Square), half on DVE (stt); squares
                # dumped into the dead z tile
                yf = y[:].rearrange("p a b c -> p (a b c)")
                hq = (3 * FREE) // 4  # ScalarE is lighter-loaded: give it more
                sq_scratch = zt[:].rearrange("p a b c -> p (a b c)")
                nc.scalar.activation(
                    out=sq_scratch[:, 0:hq], in_=yf[:, 0:hq],
                    func=mybir.ActivationFunctionType.Square,
                    accum_out=st2[:, 1:2],
                )
                nc.vector.scalar_tensor_tensor(
                    out=sq_scratch[:, hq:FREE], in0=yf[:, hq:FREE], scalar=1.0,
                    in1=yf[:, hq:FREE],
                    op0=mybir.AluOpType.mult, op1=mybir.AluOpType.mult,
                    accum_out=st2[:, 2:3],
                )

                # ---- all-reduce across partitions (GpSimd) -> every partition
                # holds (sum, sumsq); the stats math then runs replicated
                ast = stats.tile([128, 3], f32, tag="ast")
                nc.gpsimd.partition_all_reduce(
                    ast[:], st2[:], 128, bass_isa.ReduceOp.add
                )

                sm = stats.tile([128, 10], f32, tag="sm")
                mean, ex2 = sm[:, 0:1], sm[:, 1:2]
                msq, vpe = sm[:, 2:3], sm[:, 3:4]
                std, r0 = sm[:, 4:5], sm[:, 5:6]
                t1, t2 = sm[:, 6:7], sm[:, 7:8]
                t4, rr = sm[:, 8:9], sm[:, 9:10]
                nc.vector.tensor_scalar_mul(mean, ast[:, 0:1], 1.0 / NV)
                nc.vector.tensor_add(ex2, ast[:, 1:2], ast[:, 2:3])
                nc.vector.tensor_scalar_mul(ex2, ex2, 1.0 / NV)
                nc.vector.tensor_mul(msq, mean, mean)
                nc.vector.tensor_sub(vpe, ex2, msq)
                nc.vector.tensor_scalar_add(vpe, vpe, EPS)
                nc.scalar.activation(std, vpe, mybir.ActivationFunctionType.Sqrt)
                nc.vector.reciprocal(r0, std)
                # one Newton step: r = r0*(1.5 - 0.5*vpe*r0^2)
                nc.vector.tensor_mul(t1, r0, r0)
                nc.vector.tensor_mul(t2, t1, vpe)
                nc.vector.tensor_scalar(
                    t4, t2, -0.5, 1.5, op0=mybir.AluOpType.mult, op1=mybir.AluOpType.add
                )
                nc.vector.tensor_mul(rr, r0, t4)

                sb2 = stats.tile([128, 2], f32, tag="sb2")
                sc_, bi = sb2[:, 0:1], sb2[:, 1:2]
                # scale = gamma * rstd ; bias = beta - mean*scale
                nc.vector.tensor_mul(sc_, rr, gb_sb[:, p : p + 1])
                nc.vector.tensor_mul(t1, mean, sc_)
                nc.vector.tensor_sub(bi, gb_sb[:, n_pairs + p : n_pairs + p + 1], t1)

                # ---- fused normalize + ReLU (in place), then store.
                # split halves so the ScalarE apply overlaps the out-DMA
                hf = FREE // 2
                for h2 in range(2):
                    ysl = yf[:, h2 * hf : (h2 + 1) * hf]
                    nc.scalar.activation(
                        out=ysl,
                        in_=ysl,
                        func=mybir.ActivationFunctionType.Relu,
                        scale=sc_,
                        bias=bi,
                    )
                    nc.gpsimd.dma_start(
                        out=out[p][:, h2 * hf : (h2 + 1) * hf], in_=ysl
                    )

    nc.compile()
    return nc


_NC_CACHE = None


def _get_program():
    global _NC_CACHE
    if _NC_CACHE is None:
        _NC_CACHE = build_program()
    return _NC_CACHE


# Winograd F(2,3) kernel transform along kd
_G = np.array(
    [[1.0, 0.0, 0.0], [0.5, 0.5, 0.5], [0.5, -0.5, 0.5], [0.0, 0.0, 1.0]], np.float32
)


def make_core_inputs(x, w, gamma, beta, core):
    cs = slice(CH_PER_CORE * core, CH_PER_CORE * (core + 1))
    # (b, ci, d, h, w) -> pairs x (h, d, w), pair = b*8+ci; d/w zero-padded
    xp = np.zeros((N_PAIRS, D + 2, H, WP), np.float32)
    xp[:, 1 : D + 1, :, 1 : W + 1] = x[:, cs].reshape(N_PAIRS, D, H, W)
    # input transform: z_i[t] from window xpad[2t .. 2t+3] (= x[2t-1 .. 2t+2])
    a0 = xp[:, 0 : 2 * NT : 2]      # x(2t-1)
    a1 = xp[:, 1 : 2 * NT + 1 : 2]  # x(2t)
    a2 = xp[:, 2 : 2 * NT + 2 : 2]  # x(2t+1)
    a3 = xp[:, 3 : 2 * NT + 3 : 2]  # x(2t+2)
    z = np.empty((N_PAIRS, 4, NT, H, WP), np.float32)
    z[:, 0] = a0 - a2
    z[:, 1] = a1 + a2
    z[:, 2] = a2 - a1
    z[:, 3] = a1 - a3
    # -> [pair, H, point, t, WP]
    zc = np.ascontiguousarray(z.transpose(0, 3, 1, 2, 4)).astype(np.float16)

    # bands from the kd-transformed kernels ghat[i, kh, kw]
    bands = np.zeros((H, CH_PER_CORE, 4, 3, H), np.float32)
    eye0 = np.eye(H, dtype=np.float32)
    eyep = np.eye(H, k=1, dtype=np.float32)  # B[h-1, h]: kh=0 tap
    eyem = np.eye(H, k=-1, dtype=np.float32)  # B[h+1, h]: kh=2 tap
    for ci in range(CH_PER_CORE):
        c = CH_PER_CORE * core + ci
        ghat = np.einsum("ik,khw->ihw", _G, w[c, 0])  # [4, 3(kh), 3(kw)]
        for i in range(4):
            for kw in range(3):
                wk = ghat[i, :, kw]
                bands[:, ci, i, kw, :] = wk[0] * eyep + wk[1] * eye0 + wk[2] * eyem
    gbv = np.broadcast_to(
        np.concatenate([np.tile(gamma[cs], B), np.tile(beta[cs], B)])
        .astype(np.float32)
        .reshape(1, 2 * N_PAIRS),
        (128, 2 * N_PAIRS),
    ).copy()
    return {"zs": zc, "bands": bands.astype(np.float16), "gb": gbv}


def kernel(x, w, gamma, beta):
    from concourse.bass_utils import run_bass_kernel_spmd

    x = np.asarray(x, dtype=np.float32)
    w = np.asarray(w, dtype=np.float32)
    gamma = np.asarray(gamma, dtype=np.float32)
    beta = np.asarray(beta, dtype=np.float32)

    nc = _get_program()
    in_maps = [make_core_inputs(x, w, gamma, beta, k) for k in range(N_CORES)]
    res = run_bass_kernel_spmd(nc, in_maps, core_ids=list(range(N_CORES)))

    out = np.empty((B, C, D, H, W), np.float32)
    for k in range(N_CORES):
        cs = slice(CH_PER_CORE * k, CH_PER_CORE * (k + 1))
        yc = res.results[k]["out"].astype(np.float32).reshape(B, CH_PER_CORE, H, D, W)
        out[:, cs] = yc.transpose(0, 1, 3, 2, 4)
    return out
